# revision 30
# baseline (speedup 1.0000x reference)
import os
import sys

import numpy as np

for _p in ("/root/.axon_site", "/root/.axon_site/_ro/trn_rl_repo",
           "/root/.axon_site/_ro/pypackages"):
    if os.path.isdir(_p) and _p not in sys.path:
        sys.path.append(_p)

import ml_dtypes

N, C, H, W = 4, 19, 384, 384
K = 3
HP = WP = H - K + 1
N_CORES = 8
ROWS_PER_CORE = 192
SHIFTS = [(0, 0), (0, 1), (0, 2)] + [(dr, dc) for dr in (1, 2) for dc in (-2, -1, 0, 1, 2)]
NS = len(SHIFTS)
_EMIT_ORDER = [(0, 2), (0, 1), (1, 0), (1, -1), (1, 1), (1, 2), (1, -2),
               (2, 0), (2, 1), (2, 2), (2, -1), (2, -2)]
NONCENTER = [SHIFTS.index(s) for s in _EMIT_ORDER]
LGROWS = 196
P1ROWS = 68
DW0, DW1 = W, 194
GROUP_X0 = [0, 190]
COLS = 2 * NS
BCOLS = 2 * NS * 4


def _wx_profile(dc, x):
    w = np.zeros_like(x, dtype=np.float64)
    for ca in range(K):
        if 0 <= ca + dc < K:
            w += ((x - ca >= 0) & (x - ca < WP))
    return w


def _wy_profile(dr, y):
    w = np.zeros_like(y, dtype=np.float64)
    for ra in range(K):
        if 0 <= ra + dr < K:
            w += ((y - ra >= 0) & (y - ra < HP))
    return w


def _border_weights():
    bw = np.zeros((128, 2 * NS * 4), np.float64)
    for pi, DW in enumerate((DW0, DW1)):
        for si, (dr, dc) in enumerate(SHIFTS):
            wxc = sum(1 for ca in range(K) if 0 <= ca + dc < K)
            for p in range(128):
                if pi == 0:
                    gx0, own_lo, own_hi = 0, 0, W
                else:
                    g = p // 64
                    gx0 = GROUP_X0[g]
                    own_lo, own_hi = (0, 192) if g == 0 else (192, W)
                for bi, j in enumerate((0, 1, DW - 2, DW - 1)):
                    x = gx0 + j
                    if own_lo <= x < own_hi and 0 <= x + dc < W and x < W:
                        wx = _wx_profile(dc, np.array([x]))[0]
                    else:
                        wx = 0.0
                    bw[p, (pi * NS + si) * 4 + bi] = wx - wxc
    return bw


_BW = None


def _sign_maps(lbp):
    shm0 = np.zeros((128, 12, DW0), np.float32)
    shm1 = np.zeros((128, 12, DW1), np.float32)
    rows0 = np.arange(128)
    for j, si in enumerate(NONCENTER):
        dr, dc = SHIFTS[si]
        a = lbp[rows0]
        b = lbp[rows0 + dr]
        eq = np.zeros((128, W), bool)
        lo, hi = max(0, -dc), min(W, W - dc)
        eq[:, lo:hi] = a[:, lo:hi] == b[:, lo + dc:hi + dc]
        shm0[:, j, :] = np.where(eq, -1.0, 1.0)
        for g in range(2):
            x0 = GROUP_X0[g]
            rows = 128 + np.arange(64)
            a1 = lbp[rows][:, x0:x0 + DW1]
            eq1 = np.zeros((64, DW1), bool)
            xs = np.arange(x0, x0 + DW1) + dc
            ok = (xs >= 0) & (xs < W)
            eq1[:, ok] = a1[:, ok] == lbp[rows + dr][:, xs[ok]]
            shm1[64 * g:64 * g + 64, j, :] = np.where(eq1, -1.0, 1.0)
    return shm0.reshape(128, 12 * DW0), shm1.reshape(128, 12 * DW1)


def _host_inputs(logits, labels):
    in_maps = []
    for k in range(N_CORES):
        img, half = k // 2, k % 2
        g0 = half * ROWS_PER_CORE
        hi = min(H, g0 + LGROWS)
        lg = np.zeros((C, LGROWS, W), np.float32)
        lg[:, : hi - g0] = logits[img, :, g0:hi]
        lbp = np.full((LGROWS, W), -1.0, np.float32)
        lbp[: hi - g0] = labels[img, g0:hi].astype(np.float32)

        lgf = np.zeros((132, 2 + C * DW0 + 2), np.float32)
        lgf[:131, 2:2 + C * DW0] = (
            lg[:, 0:131].transpose(1, 0, 2).reshape(131, C * DW0))
        lgp1 = np.zeros((2, P1ROWS, 2 + C * DW1 + 2), np.float32)
        for g in range(2):
            x0 = GROUP_X0[g]
            lgp1[g, :, 2:2 + C * DW1] = (
                lg[:, 128:128 + P1ROWS, x0:x0 + DW1]
                .transpose(1, 0, 2).reshape(P1ROWS, C * DW1))
        shm0, shm1 = _sign_maps(lbp)
        in_maps.append({
            "lgf": lgf.astype(ml_dtypes.bfloat16),
            "lgp1": lgp1.astype(ml_dtypes.bfloat16),
            "shm0": shm0.astype(ml_dtypes.bfloat16),
            "shm1": shm1.astype(ml_dtypes.bfloat16),
        })
    return in_maps


def _combine(accs_list, bcols_list):
    global _BW
    if _BW is None:
        _BW = _border_weights()
    total = 0.0
    for k in range(N_CORES):
        acc = accs_list[k].astype(np.float64)
        bc = bcols_list[k].astype(np.float64)
        g0 = (k % 2) * ROWS_PER_CORE
        for pi in range(2):
            p = np.arange(128)
            gy = g0 + p if pi == 0 else g0 + 128 + (p % 64)
            for si, (dr, dc) in enumerate(SHIFTS):
                mult = 1.0 if (dr, dc) == (0, 0) else 2.0
                wxc = float(sum(1 for ca in range(K) if 0 <= ca + dc < K))
                wy = _wy_profile(dr, gy)
                idx = pi * NS + si
                wb = _BW[:, idx * 4: idx * 4 + 4]
                full = acc[:, idx]
                border = (bc[:, idx * 4: idx * 4 + 4] * wb).sum(1)
                total += mult * np.sum(wy * (wxc * full + border))
    return total / (N * 81 * HP * WP)


_NC = None


def _build():
    global _NC
    if _NC is not None:
        return _NC
    from concourse import bacc, mybir
    import concourse.tile as tile

    f32 = mybir.dt.float32
    bf16 = mybir.dt.bfloat16
    Alu = mybir.AluOpType
    AF = mybir.ActivationFunctionType

    from concourse.hw_specs import get_activation_tables as _gat
    _keep = "natural_log_exp_and_others"
    _mine = {AF.Exp, AF.Ln, AF.Square, AF.Copy}

    def _gat_filtered(arch):
        t = _gat(arch)
        for name in t:
            if name != _keep:
                t[name] = t[name] - _mine
        return t

    bacc.get_activation_tables = _gat_filtered

    nc = bacc.Bacc("TRN2", target_bir_lowering=False, debug=False, num_devices=N_CORES)
    lgf = nc.dram_tensor("lgf", (132, 2 + C * DW0 + 2), bf16, kind="ExternalInput")
    lgp1 = nc.dram_tensor("lgp1", (2, P1ROWS, 2 + C * DW1 + 2), bf16,
                          kind="ExternalInput")
    shm0 = nc.dram_tensor("shm0", (128, 12 * DW0), bf16, kind="ExternalInput")
    shm1 = nc.dram_tensor("shm1", (128, 12 * DW1), bf16, kind="ExternalInput")
    accs = nc.dram_tensor("accs", (128, COLS), f32, kind="ExternalOutput")
    bcols = nc.dram_tensor("bcols", (128, BCOLS), bf16, kind="ExternalOutput")

    with tile.TileContext(nc) as tc:
        with tc.tile_pool(name="persist", bufs=1) as pool, \
             tc.tile_pool(name="work", bufs=2) as wpool:
            accs_t = pool.tile([128, COLS], f32, name="accs_t")

            for pi, DW in ((1, DW1), (0, DW0)):
                FW = C * DW
                T = {}
                for dr in range(K):
                    t = pool.tile([128, FW + 4], bf16, tag=f"T{dr}_{pi}",
                                  name=f"T{dr}_{pi}")
                    if pi == 0:
                        nchunk = 8 if dr == 0 else 4
                        bnds = [round((FW + 4) * i / nchunk) for i in range(nchunk + 1)]
                        for ci, (c0, c1) in enumerate(zip(bnds[:-1], bnds[1:])):
                            eng = nc.scalar if (dr == 0 and ci % 2 == 1) else nc.sync
                            eng.dma_start(t[:, c0:c1], lgf[dr:dr + 128, c0:c1])
                    else:
                        nchunk = 4 if dr == 0 else 2
                        FW1 = FW + 4
                        bnds = [round(FW1 * i / nchunk) for i in range(nchunk + 1)]
                        for g in range(2):
                            for ci, (c0, c1) in enumerate(zip(bnds[:-1], bnds[1:])):
                                eng = nc.scalar if ci % 2 == 1 else nc.sync
                                eng.dma_start(t[64 * g:64 * g + 64, c0:c1],
                                              lgp1[g, dr:dr + 64, c0:c1])
                    T[dr] = t

                shm_t = pool.tile([128, 12 * DW], bf16, tag=f"shm_{pi}",
                                  name=f"shm_{pi}")
                shmd = shm0 if pi == 0 else shm1
                half = 6 * DW
                nc.sync.dma_start(shm_t[:, 0:half], shmd[:, 0:half])
                nc.sync.dma_start(shm_t[:, half:], shmd[:, half:])

                l1 = pool.tile([128, NS, DW], bf16, tag=f"l1_{pi}", name=f"l1_{pi}")

                first_dve = [pi == 1]

                def tree(pb3, corr3):
                    nc.vector.tensor_tensor(pb3[:, :, 0:8 * DW],
                                            pb3[:, :, 0:8 * DW],
                                            pb3[:, :, 8 * DW:16 * DW], Alu.add)
                    nc.vector.tensor_tensor(pb3[:, :, 0:4 * DW],
                                            pb3[:, :, 0:4 * DW],
                                            pb3[:, :, 4 * DW:8 * DW], Alu.add)
                    nc.vector.tensor_tensor(pb3[:, :, 0:3 * DW],
                                            pb3[:, :, 0:3 * DW],
                                            pb3[:, :, 16 * DW:19 * DW], Alu.add)
                    nc.vector.tensor_tensor(pb3[:, :, 0:2 * DW],
                                            pb3[:, :, 0:2 * DW],
                                            pb3[:, :, 2 * DW:4 * DW], Alu.add)
                    nc.vector.tensor_tensor(corr3, pb3[:, :, 0:DW],
                                            pb3[:, :, DW:2 * DW], Alu.add)

                def emit_center():
                    si = SHIFTS.index((0, 0))
                    idx = pi * NS + si
                    pb = wpool.tile([128, 1, FW], bf16, tag="pbc", bufs=1,
                                    name=f"pb_{pi}_c")
                    h = (FW // 2) & ~1
                    nc.scalar.activation(pb[:, 0, 0:h], T[0][:, 2:2 + h],
                                         AF.Square)
                    nc.scalar.activation(pb[:, 0, h:FW], T[0][:, 2 + h:2 + FW],
                                         AF.Square)
                    corr = wpool.tile([128, DW], bf16, tag="corrc", bufs=2,
                                      name=f"corr_{pi}_c")
                    tree(pb, corr[:, :].rearrange("p (s x) -> p s x", s=1))
                    u = wpool.tile([128, DW], f32, tag="uc", bufs=2,
                                   name=f"u_{pi}_c")
                    nc.scalar.activation(u[:, :], corr[:, :], AF.Exp, scale=-1.0)
                    nc.scalar.activation(
                        l1[:, si, :], u[:, :], AF.Ln, bias=1.0,
                        accum_out=accs_t[:, idx:idx + 1])

                def emit_group(j0, ng):
                    sis = NONCENTER[j0:j0 + ng]
                    pbp = wpool.tile([128, ng, FW], bf16, tag="pbp", bufs=1,
                                     name=f"pbp_{pi}_{j0}")
                    for g, si in enumerate(sis):
                        dr, dc = SHIFTS[si]
                        o1 = 2 + dc
                        splits = 2 if first_dve[0] else 1
                        first_dve[0] = False
                        bnds = [FW * i // splits for i in range(splits + 1)]
                        for c0, c1 in zip(bnds[:-1], bnds[1:]):
                            nc.vector.tensor_tensor(pbp[:, g, c0:c1],
                                                    T[0][:, 2 + c0:2 + c1],
                                                    T[dr][:, o1 + c0:o1 + c1],
                                                    Alu.mult)
                    corr = wpool.tile([128, ng * DW], bf16, tag="corrp", bufs=2,
                                      name=f"corrp_{pi}_{j0}")
                    tree(pbp, corr[:, :].rearrange("p (s x) -> p s x", s=ng))
                    wt = wpool.tile([128, ng * DW], bf16, tag="wtp", bufs=2,
                                    name=f"wtp_{pi}_{j0}")
                    nc.vector.tensor_tensor(wt[:, :],
                                            shm_t[:, j0 * DW:(j0 + ng) * DW],
                                            corr[:, :], Alu.mult)
                    u = wpool.tile([128, ng * DW], f32, tag="up", bufs=2,
                                   name=f"up_{pi}_{j0}")
                    nc.scalar.activation(u[:, :], wt[:, :], AF.Exp)
                    for g, si in enumerate(sis):
                        idx = pi * NS + si
                        nc.scalar.activation(
                            l1[:, si, :], u[:, g * DW:(g + 1) * DW], AF.Ln,
                            bias=1.0, accum_out=accs_t[:, idx:idx + 1])

                emit_center()
                for j0 in range(0, 12, 4):
                    emit_group(j0, 4)

                bc = pool.tile([128, NS, 4], bf16, tag=f"bc_{pi}", name=f"bc_{pi}")
                nc.scalar.copy(bc[:, :, 0:2], l1[:, :, 0:2])
                nc.scalar.copy(bc[:, :, 2:4], l1[:, :, DW - 2:DW])
                nc.sync.dma_start(
                    bcols[:, pi * NS * 4:(pi + 1) * NS * 4],
                    bc[:, :, :].rearrange("p s b -> p (s b)"))
                nc.sync.dma_start(accs[:, pi * NS:(pi + 1) * NS],
                                  accs_t[:, pi * NS:(pi + 1) * NS])

    nc.finalize()
    _NC = nc
    return nc


def kernel(logits, labels):
    nc = _build()
    in_maps = _host_inputs(np.asarray(logits, np.float32), np.asarray(labels))
    from concourse.bass_utils import run_bass_kernel_spmd
    res = run_bass_kernel_spmd(nc, in_maps, core_ids=list(range(N_CORES)))
    accs_list = [res.results[k]["accs"] for k in range(N_CORES)]
    bcols_list = [res.results[k]["bcols"] for k in range(N_CORES)]
    return np.array(_combine(accs_list, bcols_list), np.float32)


# revision 31
# speedup vs baseline: 1.2056x; 1.2056x over previous
import os
import sys

import numpy as np

for _p in ("/root/.axon_site", "/root/.axon_site/_ro/trn_rl_repo",
           "/root/.axon_site/_ro/pypackages"):
    if os.path.isdir(_p) and _p not in sys.path:
        sys.path.append(_p)

import ml_dtypes

N, C, H, W = 4, 19, 384, 384
K = 3
HP = WP = H - K + 1
N_CORES = 8
ROWS_PER_CORE = 192
SHIFTS = [(0, 0), (0, 1), (0, 2)] + [(dr, dc) for dr in (1, 2) for dc in (-2, -1, 0, 1, 2)]
NS = len(SHIFTS)
_EMIT_ORDER = [(0, 2), (0, 1), (1, 0), (1, -1), (1, 1), (1, 2), (1, -2),
               (2, 0), (2, 1), (2, 2), (2, -1), (2, -2)]
NONCENTER = [SHIFTS.index(s) for s in _EMIT_ORDER]
LGROWS = 196
P1ROWS = 68
DW0, DW1 = W, 194
GROUP_X0 = [0, 190]
COLS = 2 * NS
BCOLS = 2 * NS * 4


def _wx_profile(dc, x):
    w = np.zeros_like(x, dtype=np.float64)
    for ca in range(K):
        if 0 <= ca + dc < K:
            w += ((x - ca >= 0) & (x - ca < WP))
    return w


def _wy_profile(dr, y):
    w = np.zeros_like(y, dtype=np.float64)
    for ra in range(K):
        if 0 <= ra + dr < K:
            w += ((y - ra >= 0) & (y - ra < HP))
    return w


def _border_weights():
    bw = np.zeros((128, 2 * NS * 4), np.float64)
    for pi, DW in enumerate((DW0, DW1)):
        for si, (dr, dc) in enumerate(SHIFTS):
            wxc = sum(1 for ca in range(K) if 0 <= ca + dc < K)
            for p in range(128):
                if pi == 0:
                    gx0, own_lo, own_hi = 0, 0, W
                else:
                    g = p // 64
                    gx0 = GROUP_X0[g]
                    own_lo, own_hi = (0, 192) if g == 0 else (192, W)
                for bi, j in enumerate((0, 1, DW - 2, DW - 1)):
                    x = gx0 + j
                    if own_lo <= x < own_hi and 0 <= x + dc < W and x < W:
                        wx = _wx_profile(dc, np.array([x]))[0]
                    else:
                        wx = 0.0
                    bw[p, (pi * NS + si) * 4 + bi] = wx - wxc
    return bw


_BW = None


def _sign_maps(lbp):
    shm0 = np.zeros((128, 12, DW0), np.float32)
    shm1 = np.zeros((128, 12, DW1), np.float32)
    rows0 = np.arange(128)
    for j, si in enumerate(NONCENTER):
        dr, dc = SHIFTS[si]
        a = lbp[rows0]
        b = lbp[rows0 + dr]
        eq = np.zeros((128, W), bool)
        lo, hi = max(0, -dc), min(W, W - dc)
        eq[:, lo:hi] = a[:, lo:hi] == b[:, lo + dc:hi + dc]
        shm0[:, j, :] = np.where(eq, -1.0, 1.0)
        for g in range(2):
            x0 = GROUP_X0[g]
            rows = 128 + np.arange(64)
            a1 = lbp[rows][:, x0:x0 + DW1]
            eq1 = np.zeros((64, DW1), bool)
            xs = np.arange(x0, x0 + DW1) + dc
            ok = (xs >= 0) & (xs < W)
            eq1[:, ok] = a1[:, ok] == lbp[rows + dr][:, xs[ok]]
            shm1[64 * g:64 * g + 64, j, :] = np.where(eq1, -1.0, 1.0)
    return shm0.reshape(128, 12 * DW0), shm1.reshape(128, 12 * DW1)


def _host_inputs(logits, labels):
    in_maps = []
    for k in range(N_CORES):
        img, half = k // 2, k % 2
        g0 = half * ROWS_PER_CORE
        hi = min(H, g0 + LGROWS)
        lg = np.zeros((C, LGROWS, W), np.float32)
        lg[:, : hi - g0] = logits[img, :, g0:hi]
        lbp = np.full((LGROWS, W), -1.0, np.float32)
        lbp[: hi - g0] = labels[img, g0:hi].astype(np.float32)

        lgf = np.zeros((132, 2 + C * DW0 + 2), np.float32)
        lgf[:131, 2:2 + C * DW0] = (
            lg[:, 0:131].transpose(1, 0, 2).reshape(131, C * DW0))
        lgp1 = np.zeros((2, P1ROWS, 2 + C * DW1 + 2), np.float32)
        for g in range(2):
            x0 = GROUP_X0[g]
            lgp1[g, :, 2:2 + C * DW1] = (
                lg[:, 128:128 + P1ROWS, x0:x0 + DW1]
                .transpose(1, 0, 2).reshape(P1ROWS, C * DW1))
        shm0, shm1 = _sign_maps(lbp)
        in_maps.append({
            "lgf": lgf.astype(ml_dtypes.bfloat16),
            "lgp1": lgp1.astype(ml_dtypes.bfloat16),
            "shm0": shm0.astype(ml_dtypes.bfloat16),
            "shm1": shm1.astype(ml_dtypes.bfloat16),
        })
    return in_maps


def _combine(accs_list, bcols_list):
    global _BW
    if _BW is None:
        _BW = _border_weights()
    total = 0.0
    for k in range(N_CORES):
        acc = accs_list[k].astype(np.float64)
        bc = bcols_list[k].astype(np.float64)
        g0 = (k % 2) * ROWS_PER_CORE
        for pi in range(2):
            p = np.arange(128)
            gy = g0 + p if pi == 0 else g0 + 128 + (p % 64)
            for si, (dr, dc) in enumerate(SHIFTS):
                mult = 1.0 if (dr, dc) == (0, 0) else 2.0
                wxc = float(sum(1 for ca in range(K) if 0 <= ca + dc < K))
                wy = _wy_profile(dr, gy)
                idx = pi * NS + si
                wb = _BW[:, idx * 4: idx * 4 + 4]
                full = acc[:, idx]
                border = (bc[:, idx * 4: idx * 4 + 4] * wb).sum(1)
                total += mult * np.sum(wy * (wxc * full + border))
    return total / (N * 81 * HP * WP)


_NC = None


def _build():
    global _NC
    if _NC is not None:
        return _NC
    from concourse import bacc, mybir
    import concourse.tile as tile

    f32 = mybir.dt.float32
    bf16 = mybir.dt.bfloat16
    Alu = mybir.AluOpType
    AF = mybir.ActivationFunctionType

    from concourse.hw_specs import get_activation_tables as _gat
    _keep = "natural_log_exp_and_others"
    _mine = {AF.Exp, AF.Ln, AF.Square, AF.Copy}

    def _gat_filtered(arch):
        t = _gat(arch)
        for name in t:
            if name != _keep:
                t[name] = t[name] - _mine
        return t

    bacc.get_activation_tables = _gat_filtered

    nc = bacc.Bacc("TRN2", target_bir_lowering=False, debug=False, num_devices=N_CORES)
    lgf = nc.dram_tensor("lgf", (132, 2 + C * DW0 + 2), bf16, kind="ExternalInput")
    lgp1 = nc.dram_tensor("lgp1", (2, P1ROWS, 2 + C * DW1 + 2), bf16,
                          kind="ExternalInput")
    shm0 = nc.dram_tensor("shm0", (128, 12 * DW0), bf16, kind="ExternalInput")
    shm1 = nc.dram_tensor("shm1", (128, 12 * DW1), bf16, kind="ExternalInput")
    accs = nc.dram_tensor("accs", (128, COLS), f32, kind="ExternalOutput")
    bcols = nc.dram_tensor("bcols", (128, BCOLS), bf16, kind="ExternalOutput")

    with tile.TileContext(nc) as tc:
        with tc.tile_pool(name="persist", bufs=1) as pool, \
             tc.tile_pool(name="work", bufs=2) as wpool:
            accs_t = pool.tile([128, COLS], f32, name="accs_t")

            for pi, DW in ((1, DW1), (0, DW0)):
                FW = C * DW
                T = {}
                for dr in range(K):
                    t = pool.tile([128, FW + 4], bf16, tag=f"T{dr}_{pi}",
                                  name=f"T{dr}_{pi}")
                    if pi == 0:
                        nchunk = 8 if dr == 0 else 4
                        bnds = [round((FW + 4) * i / nchunk) for i in range(nchunk + 1)]
                        for ci, (c0, c1) in enumerate(zip(bnds[:-1], bnds[1:])):
                            eng = nc.scalar if (dr == 0 and ci % 2 == 1) else nc.sync
                            eng.dma_start(t[:, c0:c1], lgf[dr:dr + 128, c0:c1])
                    else:
                        nchunk = 4 if dr == 0 else 2
                        FW1 = FW + 4
                        bnds = [round(FW1 * i / nchunk) for i in range(nchunk + 1)]
                        for g in range(2):
                            for ci, (c0, c1) in enumerate(zip(bnds[:-1], bnds[1:])):
                                eng = nc.scalar if ci % 2 == 1 else nc.sync
                                eng.dma_start(t[64 * g:64 * g + 64, c0:c1],
                                              lgp1[g, dr:dr + 64, c0:c1])
                    T[dr] = t

                shm_t = pool.tile([128, 12 * DW], bf16, tag=f"shm_{pi}",
                                  name=f"shm_{pi}")
                shmd = shm0 if pi == 0 else shm1
                half = 6 * DW
                nc.sync.dma_start(shm_t[:, 0:half], shmd[:, 0:half])
                nc.sync.dma_start(shm_t[:, half:], shmd[:, half:])

                l1 = pool.tile([128, NS, DW], bf16, tag=f"l1_{pi}", name=f"l1_{pi}")

                first_dve = [pi == 1]

                def tree(pb3, corr3):
                    nc.vector.tensor_tensor(pb3[:, :, 0:8 * DW],
                                            pb3[:, :, 0:8 * DW],
                                            pb3[:, :, 8 * DW:16 * DW], Alu.add)
                    nc.vector.tensor_tensor(pb3[:, :, 0:4 * DW],
                                            pb3[:, :, 0:4 * DW],
                                            pb3[:, :, 4 * DW:8 * DW], Alu.add)
                    nc.vector.tensor_tensor(pb3[:, :, 0:3 * DW],
                                            pb3[:, :, 0:3 * DW],
                                            pb3[:, :, 16 * DW:19 * DW], Alu.add)
                    nc.vector.tensor_tensor(pb3[:, :, 0:2 * DW],
                                            pb3[:, :, 0:2 * DW],
                                            pb3[:, :, 2 * DW:4 * DW], Alu.add)
                    nc.vector.tensor_tensor(corr3, pb3[:, :, 0:DW],
                                            pb3[:, :, DW:2 * DW], Alu.add)

                def emit_center():
                    si = SHIFTS.index((0, 0))
                    idx = pi * NS + si
                    pb = wpool.tile([128, 1, FW], bf16, tag="pbc", bufs=2,
                                    name=f"pb_{pi}_c")
                    h = (FW // 2) & ~1
                    nc.scalar.activation(pb[:, 0, 0:h], T[0][:, 2:2 + h],
                                         AF.Square)
                    nc.scalar.activation(pb[:, 0, h:FW], T[0][:, 2 + h:2 + FW],
                                         AF.Square)
                    corr = wpool.tile([128, DW], bf16, tag="corrc", bufs=2,
                                      name=f"corr_{pi}_c")
                    tree(pb, corr[:, :].rearrange("p (s x) -> p s x", s=1))
                    u = wpool.tile([128, DW], f32, tag="uc", bufs=2,
                                   name=f"u_{pi}_c")
                    nc.scalar.activation(u[:, :], corr[:, :], AF.Exp, scale=-1.0)
                    nc.scalar.activation(
                        l1[:, si, :], u[:, :], AF.Ln, bias=1.0,
                        accum_out=accs_t[:, idx:idx + 1])

                def emit_group(j0, ng):
                    sis = NONCENTER[j0:j0 + ng]
                    pbp = wpool.tile([128, ng, FW], bf16, tag="pbp", bufs=2,
                                     name=f"pbp_{pi}_{j0}")
                    for g, si in enumerate(sis):
                        dr, dc = SHIFTS[si]
                        o1 = 2 + dc
                        splits = 2 if first_dve[0] else 1
                        first_dve[0] = False
                        bnds = [FW * i // splits for i in range(splits + 1)]
                        for c0, c1 in zip(bnds[:-1], bnds[1:]):
                            nc.vector.tensor_tensor(pbp[:, g, c0:c1],
                                                    T[0][:, 2 + c0:2 + c1],
                                                    T[dr][:, o1 + c0:o1 + c1],
                                                    Alu.mult)
                    corr = wpool.tile([128, ng * DW], bf16, tag="corrp", bufs=2,
                                      name=f"corrp_{pi}_{j0}")
                    tree(pbp, corr[:, :].rearrange("p (s x) -> p s x", s=ng))
                    wt = wpool.tile([128, ng * DW], bf16, tag="wtp", bufs=2,
                                    name=f"wtp_{pi}_{j0}")
                    nc.vector.tensor_tensor(wt[:, :],
                                            shm_t[:, j0 * DW:(j0 + ng) * DW],
                                            corr[:, :], Alu.mult)
                    u = wpool.tile([128, ng * DW], f32, tag="up", bufs=2,
                                   name=f"up_{pi}_{j0}")
                    nc.scalar.activation(u[:, :], wt[:, :], AF.Exp)
                    for g, si in enumerate(sis):
                        idx = pi * NS + si
                        nc.scalar.activation(
                            l1[:, si, :], u[:, g * DW:(g + 1) * DW], AF.Ln,
                            bias=1.0, accum_out=accs_t[:, idx:idx + 1])

                emit_center()
                for j0 in range(0, 12, 2):
                    emit_group(j0, 2)

                bc = pool.tile([128, NS, 4], bf16, tag=f"bc_{pi}", name=f"bc_{pi}")
                nc.scalar.copy(bc[:, :, 0:2], l1[:, :, 0:2])
                nc.scalar.copy(bc[:, :, 2:4], l1[:, :, DW - 2:DW])
                nc.sync.dma_start(
                    bcols[:, pi * NS * 4:(pi + 1) * NS * 4],
                    bc[:, :, :].rearrange("p s b -> p (s b)"))
                nc.sync.dma_start(accs[:, pi * NS:(pi + 1) * NS],
                                  accs_t[:, pi * NS:(pi + 1) * NS])

    nc.finalize()
    _NC = nc
    return nc


def kernel(logits, labels):
    nc = _build()
    in_maps = _host_inputs(np.asarray(logits, np.float32), np.asarray(labels))
    from concourse.bass_utils import run_bass_kernel_spmd
    res = run_bass_kernel_spmd(nc, in_maps, core_ids=list(range(N_CORES)))
    accs_list = [res.results[k]["accs"] for k in range(N_CORES)]
    bcols_list = [res.results[k]["bcols"] for k in range(N_CORES)]
    return np.array(_combine(accs_list, bcols_list), np.float32)
